# revision 13
# baseline (speedup 1.0000x reference)
"""Trainium2 Bass kernel for nn_DynSMHALayer (MoE-routed attention layer).

Contract: kernel(**inputs) takes FULL unsharded inputs (as produced by
reference.setup_inputs()) and returns the FULL output [B, T, C].

Sharding: 8 cores = 4 batches x 2 token-halves. Each core computes the
output for its 1024 tokens. Routing (gating) is computed on-device in
fp32; the four big projection GEMMs run in bf16 (compute-bound stage);
attention scores run in fp32, the attention value/output contractions in
fp32r.

Per-pair kv exchange: either duplicated locally (DUP_KV=True, no
collectives) or via a single AllGather over core pairs (DUP_KV=False).
"""

import math

import ml_dtypes
import numpy as np

import concourse.bacc as bacc
import concourse.bass as bass
import concourse.mybir as mybir
import concourse.tile as tile
from concourse.masks import make_identity

F32 = mybir.dt.float32
F32R = mybir.dt.float32r
BF16 = mybir.dt.bfloat16

B, T, C, D, E = 4, 2048, 2048, 128, 16
P = 128
KC = C // P              # 16 contraction chunks
NCORES = 8
T_OWN = (B * T) // NCORES  # 1024 tokens per core
NT_OWN = T_OWN // P        # 8
CH = 512                   # matmul moving-dim chunk
NCH = T_OWN // CH          # 2
T_ATT = 2 * T_OWN          # 2048 keys seen by attention
NT_ATT = T_ATT // P        # 16
SCALE = 1.0 / math.sqrt(D)
NEG_BIG = -1.0e30

# --- config knobs -----------------------------------------------------------
DUP_KV = True      # True: each core computes k/v for the whole batch (no collective)
MM_DT = BF16       # dtype of the 4 big projection GEMMs (BF16 or F32R)
TRACE = False      # request ntff profile from run_bass_kernel_spmd
# ----------------------------------------------------------------------------

_CACHED = {}


def _r(ap):
    """bitcast fp32 AP to fp32r for full-rate PE matmul."""
    return ap.bitcast(F32R)


def build_nc(dup_kv=DUP_KV, mm_dt=MM_DT):
    halves = 2 if dup_kv else 1
    t_loc = halves * T_OWN          # tokens gated/kv-projected locally
    nt_loc = t_loc // P

    nc = bacc.Bacc(None, target_bir_lowering=False, debug=False,
                   num_devices=NCORES)

    xt = nc.declare_dram_parameter("xt", [C, t_loc], mm_dt, isOutput=False)
    xg = nc.declare_dram_parameter("xg", [C, t_loc], F32, isOutput=False)
    wq = nc.declare_dram_parameter("wq", [E, P, KC, D], mm_dt, isOutput=False)
    wk = nc.declare_dram_parameter("wk", [E, P, KC, D], mm_dt, isOutput=False)
    wv = nc.declare_dram_parameter("wv", [E, P, KC, D], mm_dt, isOutput=False)
    wo = nc.declare_dram_parameter("wo", [E, D, C], mm_dt, isOutput=False)
    sn = nc.declare_dram_parameter("sn", [P, KC, E], F32, isOutput=False)
    negb = nc.declare_dram_parameter("negb", [P, E], F32, isOutput=False)
    qpos = nc.declare_dram_parameter("qpos", [1, T_OWN], F32, isOutput=False)
    spos = nc.declare_dram_parameter("spos", [P, NT_ATT], F32, isOutput=False)
    out = nc.declare_dram_parameter("out", [T_OWN, C], F32, isOutput=True)

    xt_r = xt.ap().rearrange("(k p) t -> p k t", p=P)
    xg_r = xg.ap().rearrange("(k p) t -> p k t", p=P)

    own0 = (halves - 1) * T_OWN     # own tokens are the LAST local half

    with tile.TileContext(nc) as tc:
        with (
            tc.tile_pool(name="consts", bufs=1) as consts,
            tc.tile_pool(name="accs", bufs=1) as accs,
            tc.tile_pool(name="gsc", bufs=1) as gsc,
            tc.tile_pool(name="dram", bufs=1, space="DRAM") as dram,
        ):
            ident = consts.tile([P, P], F32)
            make_identity(nc, ident)
            ones_f = consts.tile([P, 1], F32)
            nc.vector.memset(ones_f, 1.0)
            ones_b = consts.tile([P, 1], mm_dt)
            nc.vector.memset(ones_b, 1.0)
            sn_sb = consts.tile([P, KC, E], F32)
            nc.sync.dma_start(out=sn_sb, in_=sn.ap())
            negb_sb = consts.tile([P, E], F32)
            nc.sync.dma_start(out=negb_sb, in_=negb.ap())
            qpos_b = consts.tile([P, T_OWN], F32)
            nc.sync.dma_start(out=qpos_b, in_=qpos.ap()[0:1, :].to_broadcast([P, T_OWN]))
            spos_sb = consts.tile([P, NT_ATT], F32)
            nc.sync.dma_start(out=spos_sb, in_=spos.ap())

            # long-lived accumulators
            kT = accs.tile([P, t_loc], F32, tag="kT")       # [d, s_loc]
            vT = accs.tile([P, t_loc], F32, tag="vT")       # [d, s_loc]
            qT = accs.tile([P, T_OWN], F32, tag="qT")       # [d, t_own]
            kT_att = accs.tile([P, T_ATT], F32, tag="kT_att") if not dup_kv else kT
            v_att = accs.tile([P, NT_ATT, D], BF16, tag="v_att")  # [s%128, s//128, d]
            on_sb = accs.tile([P, T_OWN], F32, tag="on")    # O^T / l
            rwT_sb = accs.tile([E, t_loc], F32, tag="rwT")

            # gating scratch (token-partition layout, all local tiles)
            raw_sb = gsc.tile([P, nt_loc, E], F32, tag="raw")
            logit_sb = gsc.tile([P, nt_loc, E], F32, tag="logit")
            grelu_sb = gsc.tile([P, nt_loc, E], F32, tag="grelu")
            amask_sb = gsc.tile([P, nt_loc, E], F32, tag="amask")
            l2_sb = gsc.tile([P, nt_loc, E], F32, tag="l2")
            pexp_sb = gsc.tile([P, nt_loc, E], F32, tag="pexp")
            rw_sb = gsc.tile([P, nt_loc, E], F32, tag="rw")
            negM_sb = gsc.tile([P, nt_loc], F32, tag="negM")
            m1_sb = gsc.tile([P, nt_loc], F32, tag="m1")
            m2_sb = gsc.tile([P, nt_loc], F32, tag="m2")
            cnt_sb = gsc.tile([P, nt_loc], F32, tag="cnt")
            inact_sb = gsc.tile([P, nt_loc], F32, tag="inact")
            ssum_sb = gsc.tile([P, nt_loc], F32, tag="ssum")
            rinv_sb = gsc.tile([P, nt_loc], F32, tag="rinv")
            rcols_sb = gsc.tile([P, nt_loc], F32, tag="rcols")
            nsq_sb = gsc.tile([P, nt_loc], F32, tag="nsq")
            nrow_sb = gsc.tile([1, t_loc], F32, tag="nrow")
            linv_sb = gsc.tile([1, T_OWN], F32, tag="linv")
            linvb_sb = gsc.tile([P, T_OWN], F32, tag="linvb")

            rwT_d = dram.tile([E, t_loc], F32)
            norms_d = dram.tile([1, t_loc], F32)
            linv_d = dram.tile([1, T_OWN], F32)
            if not dup_kv:
                # flat pack: [ kT (P*T_OWN) | v tiles (P*NT_OWN*D) ]
                kv_in_d = dram.tile([2 * P * T_OWN], F32)
                kv_out_d = dram.tile([2, 2 * P * T_OWN], F32)

            for h in range(halves):
                h0 = h * T_OWN
                with (
                    tc.tile_pool(name="xth", bufs=1) as xth_pool,
                    tc.tile_pool(name="gstream", bufs=2) as gstream,
                    tc.tile_pool(name="gtmp", bufs=3) as gtmp,
                ):
                    xt_h = xth_pool.tile([P, KC, T_OWN], mm_dt)
                    for k in range(KC):
                        nc.sync.dma_start(out=xt_h[:, k, :],
                                          in_=xt_r[:, k, h0:h0 + T_OWN])

                    # ---- gating for this half's tokens -----------------
                    ps_small_cm = tc.tile_pool(name="ps_small", bufs=2,
                                               space="PSUM")
                    ps_small = ps_small_cm.__enter__()
                    for g in range(NT_OWN):
                        gi = h * NT_OWN + g
                        xg_t = gstream.tile([P, KC, P], F32, tag="xg_t")
                        nc.sync.dma_start(
                            out=xg_t, in_=xg_r[:, :, h0 + g * P:h0 + (g + 1) * P])
                        xsq_t = gstream.tile([P, KC, P], BF16, tag="xsq_t")
                        nc.scalar.square(xsq_t, xg_t)
                        ps_n = ps_small.tile([1, P], F32, tag="ps_n")
                        for k in range(KC):
                            nc.tensor.matmul(ps_n, ones_b, xsq_t[:, k, :],
                                             start=(k == 0), stop=(k == KC - 1))
                        nc.scalar.copy(nrow_sb[0:1, gi * P:(gi + 1) * P], ps_n)
                        ps_g = ps_small.tile([P, E], F32, tag="ps_g")
                        for k in range(KC):
                            nc.tensor.matmul(ps_g, xg_t[:, k, :], sn_sb[:, k, :],
                                             start=(k == 0), stop=(k == KC - 1))
                        nc.scalar.copy(raw_sb[:, gi, :], ps_g)

                    hsl = slice(h * NT_OWN, (h + 1) * NT_OWN)
                    nc.sync.dma_start(out=norms_d[0:1, h0:h0 + T_OWN],
                                      in_=nrow_sb[0:1, h0:h0 + T_OWN])
                    # norms back, transposed into token-partition columns
                    nsq_in = bass.AP(
                        tensor=norms_d[:].tensor, offset=norms_d[:].offset + h0,
                        ap=[[1, P], [P, NT_OWN]])
                    nc.sync.dma_start(out=nsq_sb[:, hsl], in_=nsq_in)
                    nc.scalar.sqrt(nsq_sb[:, hsl], nsq_sb[:, hsl])
                    nc.vector.reciprocal(rcols_sb[:, hsl], nsq_sb[:, hsl])

                    for g in range(NT_OWN):
                        gi = h * NT_OWN + g
                        nc.vector.scalar_tensor_tensor(
                            out=logit_sb[:, gi, :], in0=raw_sb[:, gi, :],
                            scalar=rcols_sb[:, gi:gi + 1], in1=negb_sb,
                            op0=mybir.AluOpType.mult, op1=mybir.AluOpType.add)
                    # batched over the half
                    nc.scalar.activation(grelu_sb[:, hsl, :], logit_sb[:, hsl, :],
                                         mybir.ActivationFunctionType.Relu)
                    nc.vector.tensor_reduce(negM_sb[:, hsl], grelu_sb[:, hsl, :],
                                            axis=mybir.AxisListType.X,
                                            op=mybir.AluOpType.max, negate=True)
                    nc.vector.tensor_single_scalar(amask_sb[:, hsl, :],
                                                   logit_sb[:, hsl, :], 0.0,
                                                   mybir.AluOpType.is_gt)
                    nc.vector.tensor_reduce(cnt_sb[:, hsl], amask_sb[:, hsl, :],
                                            axis=mybir.AxisListType.X,
                                            op=mybir.AluOpType.add)
                    nc.vector.tensor_single_scalar(inact_sb[:, hsl], cnt_sb[:, hsl],
                                                   0.0, mybir.AluOpType.is_equal)
                    nc.vector.tensor_reduce(m1_sb[:, hsl], logit_sb[:, hsl, :],
                                            axis=mybir.AxisListType.X,
                                            op=mybir.AluOpType.max)
                    for g in range(NT_OWN):
                        gi = h * NT_OWN + g
                        msk1 = gtmp.tile([P, E], F32, tag="msk1")
                        nc.vector.tensor_scalar(msk1, logit_sb[:, gi, :],
                                                m1_sb[:, gi:gi + 1], None,
                                                mybir.AluOpType.is_ge)
                        nc.vector.scalar_tensor_tensor(
                            out=l2_sb[:, gi, :], in0=msk1, scalar=NEG_BIG,
                            in1=logit_sb[:, gi, :],
                            op0=mybir.AluOpType.mult, op1=mybir.AluOpType.add)
                    nc.vector.tensor_reduce(m2_sb[:, hsl], l2_sb[:, hsl, :],
                                            axis=mybir.AxisListType.X,
                                            op=mybir.AluOpType.max)
                    for g in range(NT_OWN):
                        gi = h * NT_OWN + g
                        msk1 = gtmp.tile([P, E], F32, tag="msk1b")
                        nc.vector.tensor_scalar(msk1, logit_sb[:, gi, :],
                                                m1_sb[:, gi:gi + 1], None,
                                                mybir.AluOpType.is_ge)
                        msk2 = gtmp.tile([P, E], F32, tag="msk2")
                        nc.vector.tensor_scalar(msk2, l2_sb[:, gi, :],
                                                m2_sb[:, gi:gi + 1], None,
                                                mybir.AluOpType.is_ge)
                        fb = gtmp.tile([P, E], F32, tag="fb")
                        nc.vector.tensor_add(fb, msk1, msk2)
                        mask = gtmp.tile([P, E], F32, tag="mask")
                        nc.vector.scalar_tensor_tensor(
                            out=mask, in0=fb, scalar=inact_sb[:, gi:gi + 1],
                            in1=amask_sb[:, gi, :],
                            op0=mybir.AluOpType.mult, op1=mybir.AluOpType.add)
                        expg = gtmp.tile([P, E], F32, tag="expg")
                        nc.scalar.activation(expg, grelu_sb[:, gi, :],
                                             mybir.ActivationFunctionType.Exp,
                                             bias=negM_sb[:, gi:gi + 1], scale=1.0)
                        nc.vector.scalar_tensor_tensor(
                            out=pexp_sb[:, gi, :], in0=expg, scalar=1.0, in1=mask,
                            op0=mybir.AluOpType.mult, op1=mybir.AluOpType.mult,
                            accum_out=ssum_sb[:, gi:gi + 1])
                    nc.vector.reciprocal(rinv_sb[:, hsl], ssum_sb[:, hsl])
                    for g in range(NT_OWN):
                        gi = h * NT_OWN + g
                        nc.vector.tensor_scalar_mul(rw_sb[:, gi, :],
                                                    pexp_sb[:, gi, :],
                                                    rinv_sb[:, gi:gi + 1])
                        ps_t = ps_small.tile([E, P], F32, tag="ps_t")
                        nc.tensor.transpose(ps_t, rw_sb[:, gi, :], ident)
                        nc.scalar.copy(rwT_sb[:, gi * P:(gi + 1) * P], ps_t)
                    nc.sync.dma_start(out=rwT_d[:, h0:h0 + T_OWN],
                                      in_=rwT_sb[:, h0:h0 + T_OWN])
                    ps_small_cm.__exit__(None, None, None)

                    # ---- k/v (+q on own half) projections --------------
                    with (
                        tc.tile_pool(name="wz", bufs=4) as wz,
                        tc.tile_pool(name="rwbp", bufs=2) as rwbp,
                        tc.tile_pool(name="ztmp", bufs=4) as ztmp,
                        tc.tile_pool(name="ps_z", bufs=4, space="PSUM") as ps_z,
                    ):
                        projs = [(wk, kT), (wv, vT)]
                        if h == halves - 1:
                            projs.append((wq, qT))
                        for e in range(E):
                            rwb_e = rwbp.tile([P, T_OWN], F32, tag="rwb")
                            nc.sync.dma_start(
                                out=rwb_e,
                                in_=rwT_d[e:e + 1, h0:h0 + T_OWN]
                                .to_broadcast([P, T_OWN]))
                            for wparam, acc in projs:
                                w_e = wz.tile([P, KC, D], mm_dt, tag="w_e")
                                nc.sync.dma_start(out=w_e, in_=wparam.ap()[e])
                                is_q = acc is qT
                                a0 = 0 if is_q else h0
                                for ch in range(NCH):
                                    ps = ps_z.tile([P, CH], F32, tag="ps")
                                    for k in range(KC):
                                        nc.tensor.matmul(
                                            ps, w_e[:, k, :],
                                            xt_h[:, k, ch * CH:(ch + 1) * CH],
                                            start=(k == 0), stop=(k == KC - 1))
                                    dst = acc[:, a0 + ch * CH:a0 + (ch + 1) * CH]
                                    rsl = rwb_e[:, ch * CH:(ch + 1) * CH]
                                    if e == 0:
                                        nc.vector.tensor_mul(dst, ps, rsl)
                                    else:
                                        t = ztmp.tile([P, CH], F32, tag="zt")
                                        nc.vector.tensor_mul(t, ps, rsl)
                                        nc.vector.tensor_add(dst, dst, t)

                # v^T -> v tiles for this half
                with tc.tile_pool(name="ps_tr", bufs=2, space="PSUM") as ps_tr:
                    for s in range(NT_OWN):
                        st = h * NT_OWN + s
                        ps_v = ps_tr.tile([P, P], F32, tag="ps_v")
                        nc.tensor.transpose(ps_v, vT[:, st * P:(st + 1) * P], ident)
                        nc.scalar.copy(v_att[:, st, :], ps_v)

            # ---- pair exchange of k/v (collective path) ----------------
            if not dup_kv:
                nkv = P * T_OWN
                nc.sync.dma_start(
                    out=kv_in_d[0:nkv].rearrange("(p t) -> p t", p=P), in_=kT)
                nc.sync.dma_start(
                    out=kv_in_d[nkv:2 * nkv].rearrange("(p g d) -> p g d", p=P,
                                                       g=NT_OWN),
                    in_=v_att[:, 0:NT_OWN, :])
                nc.gpsimd.collective_compute(
                    "AllGather", mybir.AluOpType.bypass,
                    replica_groups=[[2 * i, 2 * i + 1] for i in range(NCORES // 2)],
                    ins=[kv_in_d[:].opt()], outs=[kv_out_d[:].opt()])
                for r in range(2):
                    nc.sync.dma_start(
                        out=kT_att[:, r * T_OWN:(r + 1) * T_OWN],
                        in_=kv_out_d[r, 0:nkv].rearrange("(p t) -> p t", p=P))
                    nc.sync.dma_start(
                        out=v_att[:, r * NT_OWN:(r + 1) * NT_OWN, :],
                        in_=kv_out_d[r, nkv:2 * nkv].rearrange(
                            "(p g d) -> p g d", p=P, g=NT_OWN))

            # ---- attention --------------------------------------------
            with (
                tc.tile_pool(name="ps_s", bufs=2, space="PSUM") as ps_sp,
                tc.tile_pool(name="ps_o", bufs=1, space="PSUM") as ps_op,
                tc.tile_pool(name="ps_l", bufs=1, space="PSUM") as ps_lp,
                tc.tile_pool(name="nmp", bufs=2) as nmp,
                tc.tile_pool(name="pp", bufs=3) as pp,
            ):
                ps_o = ps_op.tile([P, T_OWN], F32)
                ps_l = ps_lp.tile([1, T_OWN], F32)
                for s16 in range(NT_ATT):
                    ps_s = ps_sp.tile([P, T_OWN], F32, tag="ps_s")
                    for ch in range(NCH):
                        csl = slice(ch * CH, (ch + 1) * CH)
                        nc.tensor.matmul(ps_s[:, csl],
                                         kT_att[:, s16 * P:(s16 + 1) * P],
                                         qT[:, csl], start=True, stop=True)
                    nm = nmp.tile([P, T_OWN], F32, tag="nm")
                    nc.vector.tensor_scalar(nm, qpos_b, spos_sb[:, s16:s16 + 1],
                                            None, mybir.AluOpType.is_lt)
                    nc.vector.scalar_tensor_tensor(
                        out=ps_s, in0=nm, scalar=NEG_BIG, in1=ps_s,
                        op0=mybir.AluOpType.mult, op1=mybir.AluOpType.add)
                    p_sb = pp.tile([P, T_OWN], BF16, tag="p_sb")
                    nc.scalar.activation(p_sb, ps_s,
                                         mybir.ActivationFunctionType.Exp,
                                         scale=SCALE)
                    for ch in range(NCH):
                        csl = slice(ch * CH, (ch + 1) * CH)
                        nc.tensor.matmul(ps_l[:, csl], ones_b, p_sb[:, csl],
                                         start=(s16 == 0), stop=(s16 == NT_ATT - 1))
                        nc.tensor.matmul(ps_o[:, csl], v_att[:, s16, :],
                                         p_sb[:, csl],
                                         start=(s16 == 0), stop=(s16 == NT_ATT - 1))
                nc.vector.reciprocal(linv_sb, ps_l)
                nc.sync.dma_start(out=linv_d[:], in_=linv_sb)
                nc.sync.dma_start(out=linvb_sb,
                                  in_=linv_d[0:1, :].to_broadcast([P, T_OWN]))
                nc.vector.tensor_mul(on_sb, ps_o, linvb_sb)

            # ---- output projection ------------------------------------
            with (
                tc.tile_pool(name="utp", bufs=1) as utp,
                tc.tile_pool(name="rwb2p", bufs=2) as rwb2p,
                tc.tile_pool(name="wop", bufs=1) as wop,
                tc.tile_pool(name="osb", bufs=3) as osbp,
                tc.tile_pool(name="ps_out", bufs=3, space="PSUM") as ps_outp,
            ):
                ut = utp.tile([P, E, T_OWN], mm_dt)
                for e in range(E):
                    rwb2 = rwb2p.tile([P, T_OWN], F32, tag="rwb2")
                    nc.sync.dma_start(
                        out=rwb2,
                        in_=rwT_d[e:e + 1, own0:own0 + T_OWN]
                        .to_broadcast([P, T_OWN]))
                    nc.vector.tensor_mul(ut[:, e, :], on_sb, rwb2)
                CHALF = C // 2
                for chalf in range(2):
                    woh = wop.tile([P, E, CHALF], mm_dt, tag="woh")
                    for e in range(E):
                        nc.sync.dma_start(
                            out=woh[:, e, :],
                            in_=wo.ap()[e, :, chalf * CHALF:(chalf + 1) * CHALF])
                    for tt in range(NT_OWN):
                        ps = ps_outp.tile([P, CHALF], F32, tag="ps_out")
                        for e in range(E):
                            for cc in range(CHALF // CH):
                                nc.tensor.matmul(
                                    ps[:, cc * CH:(cc + 1) * CH],
                                    ut[:, e, tt * P:(tt + 1) * P],
                                    woh[:, e, cc * CH:(cc + 1) * CH],
                                    start=(e == 0), stop=(e == E - 1))
                        o_sb = osbp.tile([P, CHALF], F32, tag="o_sb")
                        nc.scalar.copy(o_sb, ps)
                        nc.sync.dma_start(
                            out=out.ap()[tt * P:(tt + 1) * P,
                                         chalf * CHALF:(chalf + 1) * CHALF],
                            in_=o_sb)
    nc.finalize()
    return nc


def _prep_host(inputs, dup_kv=DUP_KV, mm_dt=MM_DT):
    np_mm = np.float32 if mm_dt == F32R else ml_dtypes.bfloat16

    hs = np.ascontiguousarray(np.asarray(inputs["hidden_states"], dtype=np.float32))
    sim = np.asarray(inputs["sim_matrix"], dtype=np.float32)
    gates = np.asarray(inputs["gates"], dtype=np.float32)
    q_proj = np.asarray(inputs["q_proj"], dtype=np.float32)
    k_proj = np.asarray(inputs["k_proj"], dtype=np.float32)
    v_proj = np.asarray(inputs["v_proj"], dtype=np.float32)
    o_proj = np.asarray(inputs["o_proj"], dtype=np.float32)
    assert int(np.asarray(inputs["min_experts"])) == 2

    def wprep(w):  # [E, C, D] -> [E, P, KC, D]
        return np.ascontiguousarray(
            w.reshape(E, KC, P, D).transpose(0, 2, 1, 3)).astype(np_mm)

    wq_h, wk_h, wv_h = wprep(q_proj), wprep(k_proj), wprep(v_proj)
    wo_h = np.ascontiguousarray(o_proj).astype(np_mm)

    snorm = sim / np.maximum(np.linalg.norm(sim, axis=0, keepdims=True), 1e-12)
    sn_h = np.ascontiguousarray(
        snorm.reshape(KC, P, E).transpose(1, 0, 2)).astype(np.float32)
    negb_h = np.ascontiguousarray(
        np.tile(-1.0 / (1.0 + np.exp(-gates)), (P, 1))).astype(np.float32)
    spos_nat = (np.arange(NT_ATT)[None, :] * P
                + np.arange(P)[:, None]).astype(np.float32)

    common = dict(wq=wq_h, wk=wk_h, wv=wv_h, wo=wo_h, sn=sn_h, negb=negb_h)
    in_maps = []
    for core in range(NCORES):
        b, own = core // 2, core % 2
        xb = hs[b]                       # [T, C]
        own_sl = slice(own * T_OWN, (own + 1) * T_OWN)
        oth = 1 - own
        oth_sl = slice(oth * T_OWN, (oth + 1) * T_OWN)
        if dup_kv:
            xloc = np.concatenate([xb[oth_sl], xb[own_sl]], axis=0)  # other|own
            spos_h = np.concatenate(
                [spos_nat[:, oth * NT_OWN:(oth + 1) * NT_OWN],
                 spos_nat[:, own * NT_OWN:(own + 1) * NT_OWN]], axis=1)
            spos_h = np.ascontiguousarray(spos_h)
        else:
            xloc = xb[own_sl]
            spos_h = spos_nat
        xt_h = np.ascontiguousarray(xloc.T)
        qpos_h = (own * T_OWN + np.arange(T_OWN, dtype=np.float32))[None, :]
        in_maps.append(dict(
            common,
            xt=xt_h.astype(np_mm), xg=xt_h.astype(np.float32),
            qpos=np.ascontiguousarray(qpos_h),
            spos=spos_h.astype(np.float32)))
    return in_maps


def kernel(**inputs):
    from concourse.bass_utils import run_bass_kernel_spmd

    key = (DUP_KV, MM_DT)
    if key not in _CACHED:
        _CACHED[key] = build_nc(DUP_KV, MM_DT)
    nc = _CACHED[key]

    in_maps = _prep_host(inputs, DUP_KV, MM_DT)
    res = run_bass_kernel_spmd(nc, in_maps, list(range(NCORES)), trace=TRACE)
    kernel.last_results = res

    out = np.empty((B, T, C), dtype=np.float32)
    for core in range(NCORES):
        b, own = core // 2, core % 2
        out[b, own * T_OWN:(own + 1) * T_OWN, :] = res.results[core]["out"]
    return out


# revision 17
# speedup vs baseline: 1.0720x; 1.0720x over previous
"""Trainium2 Bass kernel for nn_DynSMHALayer (MoE-routed attention layer).

Contract: kernel(**inputs) takes FULL unsharded inputs (as produced by
reference.setup_inputs()) and returns the FULL output [B, T, C].

Sharding: 8 cores = 4 batches x 2 token-halves. Each core computes the
output for its 1024 tokens. Routing (gating) is computed on-device in
fp32; the four big projection GEMMs run in bf16 (compute-bound stage);
attention scores run in fp32, the attention value/output contractions in
fp32r.

Per-pair kv exchange: either duplicated locally (DUP_KV=True, no
collectives) or via a single AllGather over core pairs (DUP_KV=False).
"""

import math

import ml_dtypes
import numpy as np

import concourse.bacc as bacc
import concourse.bass as bass
import concourse.mybir as mybir
import concourse.tile as tile
from concourse.masks import make_identity

F32 = mybir.dt.float32
F32R = mybir.dt.float32r
BF16 = mybir.dt.bfloat16

B, T, C, D, E = 4, 2048, 2048, 128, 16
P = 128
KC = C // P              # 16 contraction chunks
NCORES = 8
T_OWN = (B * T) // NCORES  # 1024 tokens per core
NT_OWN = T_OWN // P        # 8
CH = 512                   # matmul moving-dim chunk
NCH = T_OWN // CH          # 2
T_ATT = 2 * T_OWN          # 2048 keys seen by attention
NT_ATT = T_ATT // P        # 16
SCALE = 1.0 / math.sqrt(D)
NEG_BIG = -1.0e30

# --- config knobs -----------------------------------------------------------
DUP_KV = False     # True: each core computes k/v for the whole batch (no collective)
MM_DT = BF16       # dtype of the 4 big projection GEMMs (BF16 or F32R)
TRACE = False      # request ntff profile from run_bass_kernel_spmd
# ----------------------------------------------------------------------------

_CACHED = {}


def _r(ap):
    """bitcast fp32 AP to fp32r for full-rate PE matmul."""
    return ap.bitcast(F32R)


def build_nc(dup_kv=DUP_KV, mm_dt=MM_DT):
    halves = 2 if dup_kv else 1
    t_loc = halves * T_OWN          # tokens gated/kv-projected locally
    nt_loc = t_loc // P

    nc = bacc.Bacc(None, target_bir_lowering=False, debug=False,
                   num_devices=NCORES)

    xt = nc.declare_dram_parameter("xt", [C, t_loc], mm_dt, isOutput=False)
    xg = nc.declare_dram_parameter("xg", [C, t_loc], F32, isOutput=False)
    wq = nc.declare_dram_parameter("wq", [E, P, KC, D], mm_dt, isOutput=False)
    wk = nc.declare_dram_parameter("wk", [E, P, KC, D], mm_dt, isOutput=False)
    wv = nc.declare_dram_parameter("wv", [E, P, KC, D], mm_dt, isOutput=False)
    wo = nc.declare_dram_parameter("wo", [E, D, C], mm_dt, isOutput=False)
    sn = nc.declare_dram_parameter("sn", [P, KC, E], F32, isOutput=False)
    negb = nc.declare_dram_parameter("negb", [P, E], F32, isOutput=False)
    qpos = nc.declare_dram_parameter("qpos", [1, T_OWN], F32, isOutput=False)
    spos = nc.declare_dram_parameter("spos", [P, NT_ATT], F32, isOutput=False)
    out = nc.declare_dram_parameter("out", [T_OWN, C], F32, isOutput=True)

    xt_r = xt.ap().rearrange("(k p) t -> p k t", p=P)
    xg_r = xg.ap().rearrange("(k p) t -> p k t", p=P)

    own0 = (halves - 1) * T_OWN     # own tokens are the LAST local half

    with tile.TileContext(nc) as tc:
        with (
            tc.tile_pool(name="consts", bufs=1) as consts,
            tc.tile_pool(name="accs", bufs=1) as accs,
            tc.tile_pool(name="gsc", bufs=1) as gsc,
            tc.tile_pool(name="dram", bufs=1, space="DRAM") as dram,
        ):
            ident = consts.tile([P, P], F32)
            make_identity(nc, ident)
            ones_f = consts.tile([P, 1], F32)
            nc.vector.memset(ones_f, 1.0)
            ones_b = consts.tile([P, 1], mm_dt)
            nc.vector.memset(ones_b, 1.0)
            sn_sb = consts.tile([P, KC, E], F32)
            nc.sync.dma_start(out=sn_sb, in_=sn.ap())
            negb_sb = consts.tile([P, E], F32)
            nc.sync.dma_start(out=negb_sb, in_=negb.ap())
            qpos_b = consts.tile([P, T_OWN], F32)
            nc.sync.dma_start(out=qpos_b, in_=qpos.ap()[0:1, :].to_broadcast([P, T_OWN]))
            spos_sb = consts.tile([P, NT_ATT], F32)
            nc.sync.dma_start(out=spos_sb, in_=spos.ap())

            # long-lived accumulators
            kT = accs.tile([P, t_loc], F32, tag="kT")       # [d, s_loc]
            vT = accs.tile([P, t_loc], F32, tag="vT")       # [d, s_loc]
            qT = accs.tile([P, T_OWN], F32, tag="qT")       # [d, t_own]
            if not dup_kv:
                kT_att = accs.tile([P, T_ATT], F32, tag="kT_att")
            else:
                kT_att = kT
            v_att = accs.tile([P, NT_ATT, D], BF16, tag="v_att")  # [s%128, s//128, d]
            on_sb = accs.tile([P, T_OWN], F32, tag="on")    # O^T / l
            rwT_sb = accs.tile([E, t_loc], F32, tag="rwT")

            # gating scratch (token-partition layout, all local tiles)
            raw_sb = gsc.tile([P, nt_loc, E], F32, tag="raw")
            logit_sb = gsc.tile([P, nt_loc, E], F32, tag="logit")
            grelu_sb = gsc.tile([P, nt_loc, E], F32, tag="grelu")
            amask_sb = gsc.tile([P, nt_loc, E], F32, tag="amask")
            l2_sb = gsc.tile([P, nt_loc, E], F32, tag="l2")
            pexp_sb = gsc.tile([P, nt_loc, E], F32, tag="pexp")
            rw_sb = gsc.tile([P, nt_loc, E], F32, tag="rw")
            negM_sb = gsc.tile([P, nt_loc], F32, tag="negM")
            m1_sb = gsc.tile([P, nt_loc], F32, tag="m1")
            m2_sb = gsc.tile([P, nt_loc], F32, tag="m2")
            cnt_sb = gsc.tile([P, nt_loc], F32, tag="cnt")
            inact_sb = gsc.tile([P, nt_loc], F32, tag="inact")
            ssum_sb = gsc.tile([P, nt_loc], F32, tag="ssum")
            rinv_sb = gsc.tile([P, nt_loc], F32, tag="rinv")
            rcols_sb = gsc.tile([P, nt_loc], F32, tag="rcols")
            nsq_sb = gsc.tile([P, nt_loc], F32, tag="nsq")
            nrow_sb = gsc.tile([1, t_loc], F32, tag="nrow")
            linv_sb = gsc.tile([1, T_OWN], F32, tag="linv")
            linvb_sb = gsc.tile([P, T_OWN], F32, tag="linvb")

            rwT_d = dram.tile([E, t_loc], F32)
            norms_d = dram.tile([1, t_loc], F32)
            linv_d = dram.tile([1, T_OWN], F32)
            if not dup_kv:
                # pack: [ kT f32 (P*T_OWN) | v bf16 bitcast to f32 (P*T_OWN/2) ]
                nkv_pack = P * T_OWN + P * T_OWN // 2
                kv_in_d = dram.tile([nkv_pack], F32)
                kv_out_d = dram.tile([2, nkv_pack], F32)

            for h in range(halves):
                h0 = h * T_OWN
                with (
                    tc.tile_pool(name="xth", bufs=1) as xth_pool,
                    tc.tile_pool(name="gstream", bufs=2) as gstream,
                    tc.tile_pool(name="gtmp", bufs=3) as gtmp,
                ):
                    xt_h = xth_pool.tile([P, KC, T_OWN], mm_dt)
                    for k in range(KC):
                        nc.sync.dma_start(out=xt_h[:, k, :],
                                          in_=xt_r[:, k, h0:h0 + T_OWN])

                    # ---- gating for this half's tokens -----------------
                    ps_small_cm = tc.tile_pool(name="ps_small", bufs=2,
                                               space="PSUM")
                    ps_small = ps_small_cm.__enter__()
                    for g in range(NT_OWN):
                        gi = h * NT_OWN + g
                        xg_t = gstream.tile([P, KC, P], F32, tag="xg_t")
                        nc.sync.dma_start(
                            out=xg_t, in_=xg_r[:, :, h0 + g * P:h0 + (g + 1) * P])
                        xsq_t = gstream.tile([P, KC, P], BF16, tag="xsq_t")
                        nc.scalar.square(xsq_t, xg_t)
                        ps_n = ps_small.tile([1, P], F32, tag="ps_n")
                        for k in range(KC):
                            nc.tensor.matmul(ps_n, ones_b, xsq_t[:, k, :],
                                             start=(k == 0), stop=(k == KC - 1))
                        nc.scalar.copy(nrow_sb[0:1, gi * P:(gi + 1) * P], ps_n)
                        ps_g = ps_small.tile([P, E], F32, tag="ps_g")
                        for k in range(KC):
                            nc.tensor.matmul(ps_g, xg_t[:, k, :], sn_sb[:, k, :],
                                             start=(k == 0), stop=(k == KC - 1))
                        nc.scalar.copy(raw_sb[:, gi, :], ps_g)

                    hsl = slice(h * NT_OWN, (h + 1) * NT_OWN)
                    nc.sync.dma_start(out=norms_d[0:1, h0:h0 + T_OWN],
                                      in_=nrow_sb[0:1, h0:h0 + T_OWN])
                    # norms back, transposed into token-partition columns
                    nsq_in = bass.AP(
                        tensor=norms_d[:].tensor, offset=norms_d[:].offset + h0,
                        ap=[[1, P], [P, NT_OWN]])
                    nc.sync.dma_start(out=nsq_sb[:, hsl], in_=nsq_in)
                    nc.scalar.sqrt(nsq_sb[:, hsl], nsq_sb[:, hsl])
                    nc.vector.reciprocal(rcols_sb[:, hsl], nsq_sb[:, hsl])

                    for g in range(NT_OWN):
                        gi = h * NT_OWN + g
                        nc.vector.scalar_tensor_tensor(
                            out=logit_sb[:, gi, :], in0=raw_sb[:, gi, :],
                            scalar=rcols_sb[:, gi:gi + 1], in1=negb_sb,
                            op0=mybir.AluOpType.mult, op1=mybir.AluOpType.add)
                    # batched over the half
                    nc.scalar.activation(grelu_sb[:, hsl, :], logit_sb[:, hsl, :],
                                         mybir.ActivationFunctionType.Relu)
                    nc.vector.tensor_reduce(negM_sb[:, hsl], grelu_sb[:, hsl, :],
                                            axis=mybir.AxisListType.X,
                                            op=mybir.AluOpType.max, negate=True)
                    nc.vector.tensor_single_scalar(amask_sb[:, hsl, :],
                                                   logit_sb[:, hsl, :], 0.0,
                                                   mybir.AluOpType.is_gt)
                    nc.vector.tensor_reduce(cnt_sb[:, hsl], amask_sb[:, hsl, :],
                                            axis=mybir.AxisListType.X,
                                            op=mybir.AluOpType.add)
                    nc.vector.tensor_single_scalar(inact_sb[:, hsl], cnt_sb[:, hsl],
                                                   0.0, mybir.AluOpType.is_equal)
                    nc.vector.tensor_reduce(m1_sb[:, hsl], logit_sb[:, hsl, :],
                                            axis=mybir.AxisListType.X,
                                            op=mybir.AluOpType.max)
                    for g in range(NT_OWN):
                        gi = h * NT_OWN + g
                        msk1 = gtmp.tile([P, E], F32, tag="msk1")
                        nc.vector.tensor_scalar(msk1, logit_sb[:, gi, :],
                                                m1_sb[:, gi:gi + 1], None,
                                                mybir.AluOpType.is_ge)
                        nc.vector.scalar_tensor_tensor(
                            out=l2_sb[:, gi, :], in0=msk1, scalar=NEG_BIG,
                            in1=logit_sb[:, gi, :],
                            op0=mybir.AluOpType.mult, op1=mybir.AluOpType.add)
                    nc.vector.tensor_reduce(m2_sb[:, hsl], l2_sb[:, hsl, :],
                                            axis=mybir.AxisListType.X,
                                            op=mybir.AluOpType.max)
                    for g in range(NT_OWN):
                        gi = h * NT_OWN + g
                        msk1 = gtmp.tile([P, E], F32, tag="msk1b")
                        nc.vector.tensor_scalar(msk1, logit_sb[:, gi, :],
                                                m1_sb[:, gi:gi + 1], None,
                                                mybir.AluOpType.is_ge)
                        msk2 = gtmp.tile([P, E], F32, tag="msk2")
                        nc.vector.tensor_scalar(msk2, l2_sb[:, gi, :],
                                                m2_sb[:, gi:gi + 1], None,
                                                mybir.AluOpType.is_ge)
                        fb = gtmp.tile([P, E], F32, tag="fb")
                        nc.vector.tensor_add(fb, msk1, msk2)
                        mask = gtmp.tile([P, E], F32, tag="mask")
                        nc.vector.scalar_tensor_tensor(
                            out=mask, in0=fb, scalar=inact_sb[:, gi:gi + 1],
                            in1=amask_sb[:, gi, :],
                            op0=mybir.AluOpType.mult, op1=mybir.AluOpType.add)
                        expg = gtmp.tile([P, E], F32, tag="expg")
                        nc.scalar.activation(expg, grelu_sb[:, gi, :],
                                             mybir.ActivationFunctionType.Exp,
                                             bias=negM_sb[:, gi:gi + 1], scale=1.0)
                        nc.vector.scalar_tensor_tensor(
                            out=pexp_sb[:, gi, :], in0=expg, scalar=1.0, in1=mask,
                            op0=mybir.AluOpType.mult, op1=mybir.AluOpType.mult,
                            accum_out=ssum_sb[:, gi:gi + 1])
                    nc.vector.reciprocal(rinv_sb[:, hsl], ssum_sb[:, hsl])
                    for g in range(NT_OWN):
                        gi = h * NT_OWN + g
                        nc.vector.tensor_scalar_mul(rw_sb[:, gi, :],
                                                    pexp_sb[:, gi, :],
                                                    rinv_sb[:, gi:gi + 1])
                        ps_t = ps_small.tile([E, P], F32, tag="ps_t")
                        nc.tensor.transpose(ps_t, rw_sb[:, gi, :], ident)
                        nc.scalar.copy(rwT_sb[:, gi * P:(gi + 1) * P], ps_t)
                    nc.sync.dma_start(out=rwT_d[:, h0:h0 + T_OWN],
                                      in_=rwT_sb[:, h0:h0 + T_OWN])
                    ps_small_cm.__exit__(None, None, None)

                    # ---- k/v (+q on own half) projections --------------
                    with (
                        tc.tile_pool(name="wz", bufs=4) as wz,
                        tc.tile_pool(name="rwbp", bufs=2) as rwbp,
                        tc.tile_pool(name="ztmp", bufs=4) as ztmp,
                        tc.tile_pool(name="ps_z", bufs=4, space="PSUM") as ps_z,
                    ):
                        projs = [(wk, kT), (wv, vT)]
                        if h == halves - 1:
                            projs.append((wq, qT))
                        for e in range(E):
                            rwb_e = rwbp.tile([P, T_OWN], F32, tag="rwb")
                            nc.sync.dma_start(
                                out=rwb_e,
                                in_=rwT_d[e:e + 1, h0:h0 + T_OWN]
                                .to_broadcast([P, T_OWN]))
                            for wparam, acc in projs:
                                w_e = wz.tile([P, KC, D], mm_dt, tag="w_e")
                                nc.sync.dma_start(out=w_e, in_=wparam.ap()[e])
                                is_q = acc is qT
                                a0 = 0 if is_q else h0
                                for ch in range(NCH):
                                    ps = ps_z.tile([P, CH], F32, tag="ps")
                                    for k in range(KC):
                                        nc.tensor.matmul(
                                            ps, w_e[:, k, :],
                                            xt_h[:, k, ch * CH:(ch + 1) * CH],
                                            start=(k == 0), stop=(k == KC - 1))
                                    dst = acc[:, a0 + ch * CH:a0 + (ch + 1) * CH]
                                    rsl = rwb_e[:, ch * CH:(ch + 1) * CH]
                                    if e == 0:
                                        nc.vector.tensor_mul(dst, ps, rsl)
                                    else:
                                        t = ztmp.tile([P, CH], F32, tag="zt")
                                        nc.vector.tensor_mul(t, ps, rsl)
                                        nc.vector.tensor_add(dst, dst, t)

                # v^T -> v tiles for this half
                with tc.tile_pool(name="ps_tr", bufs=2, space="PSUM") as ps_tr:
                    for s in range(NT_OWN):
                        st = h * NT_OWN + s
                        ps_v = ps_tr.tile([P, P], F32, tag="ps_v")
                        nc.tensor.transpose(ps_v, vT[:, st * P:(st + 1) * P], ident)
                        nc.scalar.copy(v_att[:, st, :], ps_v)

            # ---- pair exchange of k/v (collective path) ----------------
            if not dup_kv:
                nk = P * T_OWN
                nc.sync.dma_start(
                    out=kv_in_d[0:nk].rearrange("(p t) -> p t", p=P), in_=kT)
                nc.sync.dma_start(
                    out=kv_in_d[nk:nkv_pack].rearrange("(p g d) -> p g d",
                                                       p=P, g=NT_OWN),
                    in_=v_att[:, 0:NT_OWN, :].bitcast(F32))
                nc.gpsimd.collective_compute(
                    "AllGather", mybir.AluOpType.bypass,
                    replica_groups=[[2 * i, 2 * i + 1] for i in range(NCORES // 2)],
                    ins=[kv_in_d[:].opt()], outs=[kv_out_d[:].opt()])
                for r in range(2):
                    nc.sync.dma_start(
                        out=kT_att[:, r * T_OWN:(r + 1) * T_OWN],
                        in_=kv_out_d[r, 0:nk].rearrange("(p t) -> p t", p=P))
                    nc.sync.dma_start(
                        out=v_att[:, r * NT_OWN:(r + 1) * NT_OWN, :].bitcast(F32),
                        in_=kv_out_d[r, nk:nkv_pack].rearrange(
                            "(p g d) -> p g d", p=P, g=NT_OWN))

            # ---- attention --------------------------------------------
            with (
                tc.tile_pool(name="ps_s", bufs=2, space="PSUM") as ps_sp,
                tc.tile_pool(name="ps_o", bufs=1, space="PSUM") as ps_op,
                tc.tile_pool(name="ps_l", bufs=1, space="PSUM") as ps_lp,
                tc.tile_pool(name="nmp", bufs=2) as nmp,
                tc.tile_pool(name="pp", bufs=3) as pp,
            ):
                ps_o = ps_op.tile([P, T_OWN], F32)
                ps_l = ps_lp.tile([1, T_OWN], F32)
                for s16 in range(NT_ATT):
                    ps_s = ps_sp.tile([P, T_OWN], F32, tag="ps_s")
                    for ch in range(NCH):
                        csl = slice(ch * CH, (ch + 1) * CH)
                        nc.tensor.matmul(ps_s[:, csl],
                                         kT_att[:, s16 * P:(s16 + 1) * P],
                                         qT[:, csl], start=True, stop=True)
                    nm = nmp.tile([P, T_OWN], F32, tag="nm")
                    nc.vector.tensor_scalar(nm, qpos_b, spos_sb[:, s16:s16 + 1],
                                            None, mybir.AluOpType.is_lt)
                    nc.vector.scalar_tensor_tensor(
                        out=ps_s, in0=nm, scalar=NEG_BIG, in1=ps_s,
                        op0=mybir.AluOpType.mult, op1=mybir.AluOpType.add)
                    p_sb = pp.tile([P, T_OWN], BF16, tag="p_sb")
                    nc.scalar.activation(p_sb, ps_s,
                                         mybir.ActivationFunctionType.Exp,
                                         scale=SCALE)
                    for ch in range(NCH):
                        csl = slice(ch * CH, (ch + 1) * CH)
                        nc.tensor.matmul(ps_l[:, csl], ones_b, p_sb[:, csl],
                                         start=(s16 == 0), stop=(s16 == NT_ATT - 1))
                        nc.tensor.matmul(ps_o[:, csl], v_att[:, s16, :],
                                         p_sb[:, csl],
                                         start=(s16 == 0), stop=(s16 == NT_ATT - 1))
                nc.vector.reciprocal(linv_sb, ps_l)
                nc.sync.dma_start(out=linv_d[:], in_=linv_sb)
                nc.sync.dma_start(out=linvb_sb,
                                  in_=linv_d[0:1, :].to_broadcast([P, T_OWN]))
                nc.vector.tensor_mul(on_sb, ps_o, linvb_sb)

            # ---- output projection ------------------------------------
            with (
                tc.tile_pool(name="utp", bufs=1) as utp,
                tc.tile_pool(name="rwb2p", bufs=2) as rwb2p,
                tc.tile_pool(name="wop", bufs=1) as wop,
                tc.tile_pool(name="osb", bufs=3) as osbp,
                tc.tile_pool(name="ps_out", bufs=3, space="PSUM") as ps_outp,
            ):
                ut = utp.tile([P, E, T_OWN], mm_dt)
                for e in range(E):
                    rwb2 = rwb2p.tile([P, T_OWN], F32, tag="rwb2")
                    nc.sync.dma_start(
                        out=rwb2,
                        in_=rwT_d[e:e + 1, own0:own0 + T_OWN]
                        .to_broadcast([P, T_OWN]))
                    nc.vector.tensor_mul(ut[:, e, :], on_sb, rwb2)
                CHALF = C // 2
                for chalf in range(2):
                    woh = wop.tile([P, E, CHALF], mm_dt, tag="woh")
                    for e in range(E):
                        nc.sync.dma_start(
                            out=woh[:, e, :],
                            in_=wo.ap()[e, :, chalf * CHALF:(chalf + 1) * CHALF])
                    for tt in range(NT_OWN):
                        ps = ps_outp.tile([P, CHALF], F32, tag="ps_out")
                        for e in range(E):
                            for cc in range(CHALF // CH):
                                nc.tensor.matmul(
                                    ps[:, cc * CH:(cc + 1) * CH],
                                    ut[:, e, tt * P:(tt + 1) * P],
                                    woh[:, e, cc * CH:(cc + 1) * CH],
                                    start=(e == 0), stop=(e == E - 1))
                        o_sb = osbp.tile([P, CHALF], F32, tag="o_sb")
                        nc.scalar.copy(o_sb, ps)
                        nc.sync.dma_start(
                            out=out.ap()[tt * P:(tt + 1) * P,
                                         chalf * CHALF:(chalf + 1) * CHALF],
                            in_=o_sb)
    nc.finalize()
    return nc


def _prep_host(inputs, dup_kv=DUP_KV, mm_dt=MM_DT):
    np_mm = np.float32 if mm_dt == F32R else ml_dtypes.bfloat16

    hs = np.ascontiguousarray(np.asarray(inputs["hidden_states"], dtype=np.float32))
    sim = np.asarray(inputs["sim_matrix"], dtype=np.float32)
    gates = np.asarray(inputs["gates"], dtype=np.float32)
    q_proj = np.asarray(inputs["q_proj"], dtype=np.float32)
    k_proj = np.asarray(inputs["k_proj"], dtype=np.float32)
    v_proj = np.asarray(inputs["v_proj"], dtype=np.float32)
    o_proj = np.asarray(inputs["o_proj"], dtype=np.float32)
    assert int(np.asarray(inputs["min_experts"])) == 2

    def wprep(w):  # [E, C, D] -> [E, P, KC, D]
        return np.ascontiguousarray(
            w.reshape(E, KC, P, D).transpose(0, 2, 1, 3)).astype(np_mm)

    wq_h, wk_h, wv_h = wprep(q_proj), wprep(k_proj), wprep(v_proj)
    wo_h = np.ascontiguousarray(o_proj).astype(np_mm)

    snorm = sim / np.maximum(np.linalg.norm(sim, axis=0, keepdims=True), 1e-12)
    sn_h = np.ascontiguousarray(
        snorm.reshape(KC, P, E).transpose(1, 0, 2)).astype(np.float32)
    negb_h = np.ascontiguousarray(
        np.tile(-1.0 / (1.0 + np.exp(-gates)), (P, 1))).astype(np.float32)
    spos_nat = (np.arange(NT_ATT)[None, :] * P
                + np.arange(P)[:, None]).astype(np.float32)

    common = dict(wq=wq_h, wk=wk_h, wv=wv_h, wo=wo_h, sn=sn_h, negb=negb_h)
    in_maps = []
    for core in range(NCORES):
        b, own = core // 2, core % 2
        xb = hs[b]                       # [T, C]
        own_sl = slice(own * T_OWN, (own + 1) * T_OWN)
        oth = 1 - own
        oth_sl = slice(oth * T_OWN, (oth + 1) * T_OWN)
        if dup_kv:
            xloc = np.concatenate([xb[oth_sl], xb[own_sl]], axis=0)  # other|own
            spos_h = np.concatenate(
                [spos_nat[:, oth * NT_OWN:(oth + 1) * NT_OWN],
                 spos_nat[:, own * NT_OWN:(own + 1) * NT_OWN]], axis=1)
            spos_h = np.ascontiguousarray(spos_h)
        else:
            xloc = xb[own_sl]
            spos_h = spos_nat
        xt_h = np.ascontiguousarray(xloc.T)
        qpos_h = (own * T_OWN + np.arange(T_OWN, dtype=np.float32))[None, :]
        in_maps.append(dict(
            common,
            xt=xt_h.astype(np_mm), xg=xt_h.astype(np.float32),
            qpos=np.ascontiguousarray(qpos_h),
            spos=spos_h.astype(np.float32)))
    return in_maps


def kernel(**inputs):
    from concourse.bass_utils import run_bass_kernel_spmd

    key = (DUP_KV, MM_DT)
    if key not in _CACHED:
        _CACHED[key] = build_nc(DUP_KV, MM_DT)
    nc = _CACHED[key]

    in_maps = _prep_host(inputs, DUP_KV, MM_DT)
    res = run_bass_kernel_spmd(nc, in_maps, list(range(NCORES)), trace=TRACE)
    kernel.last_results = res

    out = np.empty((B, T, C), dtype=np.float32)
    for core in range(NCORES):
        b, own = core // 2, core % 2
        out[b, own * T_OWN:(own + 1) * T_OWN, :] = res.results[core]["out"]
    return out
